# revision 22
# baseline (speedup 1.0000x reference)
"""Trainium2 Bass kernel for nn_FSMNSeleNetV3 (FSMN stack + channel maxpool + decoder).

Self-contained: hardcodes all shapes from the problem spec and only imports
numpy + the concourse stack from /opt/trn_rl_repo.

Sharding: pure data parallel over batch. Each of the 8 cores processes 4
batches x 4 channels = 16 independent sequences of T=2048 tokens.

v3 design:
- fp16 everywhere on the matmul path (fp32r lowers to fp32_mode=HIGH which is
  ~2x slower per column AND disables FWL; 16-bit gets 1 cycle/row + fast
  weight load, and fp16's 11-bit mantissa keeps precision).
- The shrink writes both T-halves into window-granular 1-bank PSUM tiles via
  column tiling (tile_position (0,0)/(0,64)) -> the halves-stacked h layout.
- FSMN conv: taps delta -9..-2 as 128x128 block-diag fp16 matmuls on the PE
  accumulating into the shrink PSUM (DVE scalar_tensor_tensor only has a
  1x-mode uop, so the PE is the cheapest tap engine); taps delta -1,0,+1 as
  a DVE chain; per-window merge (s + pcs) on the DVE.
- Halo copies and the channel maxpool run on the otherwise-idle GpSimd.
- 4-channel software pipelining per unit plus cross-batch overlap (pool +
  decoder of batch b are emitted after batch b+1's unit-0 stages) to keep
  the PE saturated so the HAM clock gate stays at 2.4 GHz.
"""

import sys

sys.path.insert(0, "/opt/trn_rl_repo")
from contextlib import ExitStack

import numpy as np

import concourse.bass as bass  # noqa: F401
import concourse.mybir as mybir
import concourse.tile as tile
from concourse import bacc
from concourse.bass_utils import run_bass_kernel_spmd

F32 = mybir.dt.float32
F16 = mybir.dt.float16
AF = mybir.ActivationFunctionType
OP = mybir.AluOpType

NCORES = 8
B, T, C, F = 32, 2048, 4, 120
DL, DP, L, LO, RO, S = 128, 64, 5, 10, 1, 5
BPC = B // NCORES  # batches per core
SEQ = BPC * C  # sequences per core
H = T // 2  # half-sequence length (halves stacked on partitions)
HALO_L = LO - 1  # 9 left halo columns
HW = H + HALO_L + RO  # h buffer width: 1034

# conv tap split per unit: tap index j = delta + 9, delta in [-9..+1].
# PE taps run as block-diag matmuls; Act taps as scaled copies (per-partition
# scale AP) summed by the DVE; DVE taps as an stt chain (seed carries the
# residual). Units 3-4 shift one tap PE->DVE to balance engine load.
PE_TAPS_L = {l: ([0, 1, 2, 3, 4, 5, 6] if l < 3 else [0, 1, 2, 3, 4, 5]) for l in range(L)}
ACT_TAPS_L = {l: [7, 10] for l in range(L)}
DVE_TAPS_L = {l: ([8, 9] if l < 3 else [6, 8, 9]) for l in range(L)}
NPE_MAX = 7  # diag table stride (max PE taps of any unit)


def build_nc():
    nc = bacc.Bacc("TRN2", target_bir_lowering=False, debug=False, num_devices=NCORES)

    xt_d = nc.dram_tensor("xt", [SEQ, F, T], F16, kind="ExternalInput")
    we0_d = nc.dram_tensor("we0", [F, DL], F16, kind="ExternalInput")
    wedup_d = nc.dram_tensor("wedup", [L, 2 * DP, DL], F16, kind="ExternalInput")
    ws_d = nc.dram_tensor("ws", [L, DL, DP], F16, kind="ExternalInput")
    wd_d = nc.dram_tensor("wd", [DL, S], F16, kind="ExternalInput")
    biases_d = nc.dram_tensor("biases", [DL, L + 1], F32, kind="ExternalInput")
    taps_d = nc.dram_tensor("taps", [2 * DP, L * 11], F32, kind="ExternalInput")
    diag_d = nc.dram_tensor(
        "diag", [2 * DP, L * NPE_MAX * 2 * DP], F16, kind="ExternalInput"
    )
    bd_d = nc.dram_tensor("bd", [S, 1], F32, kind="ExternalInput")
    out_d = nc.dram_tensor("out", [BPC, S, T], F32, kind="ExternalOutput")

    with tile.TileContext(nc) as tc, ExitStack() as ctx:
        wp = ctx.enter_context(tc.tile_pool(name="weights", bufs=1))
        xp = ctx.enter_context(tc.tile_pool(name="x", bufs=9))
        ep = ctx.enter_context(tc.tile_pool(name="e", bufs=5))
        hp = ctx.enter_context(tc.tile_pool(name="h", bufs=4))
        sp_ = ctx.enter_context(tc.tile_pool(name="s", bufs=4))
        tp_ = ctx.enter_context(tc.tile_pool(name="tmp", bufs=10))
        op_ = ctx.enter_context(tc.tile_pool(name="o", bufs=10))
        fp = ctx.enter_context(tc.tile_pool(name="f", bufs=10))
        pp = ctx.enter_context(tc.tile_pool(name="pooled", bufs=4))
        osb = ctx.enter_context(tc.tile_pool(name="osb", bufs=1))
        pse = ctx.enter_context(tc.tile_pool(name="pse", bufs=2, space="PSUM"))
        psc = ctx.enter_context(tc.tile_pool(name="psc", bufs=4, space="PSUM"))

        # --- weights / constants (loaded once) ---
        we0_sb = wp.tile([F, DL], F16)
        nc.sync.dma_start(out=we0_sb[:], in_=we0_d[:])
        wedup_sb = wp.tile([2 * DP, L * DL], F16)
        ws_sb = wp.tile([DL, L * DP], F16)
        for l in range(L):
            nc.sync.dma_start(out=wedup_sb[:, l * DL : (l + 1) * DL], in_=wedup_d[l])
            nc.sync.dma_start(out=ws_sb[:, l * DP : (l + 1) * DP], in_=ws_d[l])
        wd_sb = wp.tile([DL, S], F16)
        nc.sync.dma_start(out=wd_sb[:], in_=wd_d[:])
        bias_sb = wp.tile([DL, L + 1], F32)
        nc.sync.dma_start(out=bias_sb[:], in_=biases_d[:])
        taps_sb = wp.tile([2 * DP, L * 11], F32)
        nc.sync.dma_start(out=taps_sb[:], in_=taps_d[:])
        diag_sb = wp.tile([2 * DP, L * NPE_MAX * 2 * DP], F16)
        bd_sb = wp.tile([S, 1], F32)
        nc.sync.dma_start(out=bd_sb[:], in_=bd_d[:])
        zero_sb = wp.tile([2 * DP, HALO_L], F16)
        nc.gpsimd.memset(zero_sb[:], 0.0)

        def tap(l, j):
            return taps_sb[:, l * 11 + j : l * 11 + j + 1]

        def diag(l, i):
            col = (l * NPE_MAX + i) * 2 * DP
            return diag_sb[:, col : col + 2 * DP]

        def stage_expand(l, src):
            """Expand + relu -> e_sb [128, T] fp16 (plain-T layout).

            l==0: src is x_sb [120, T].  1<=l<=4: src is o [2x64 halves, H].
            l==5: final expand with We2."""
            e_sb = ep.tile([DL, T], F16)
            bcol = 0 if l == 0 else l
            if l == 0:
                for g in range(2):
                    pe = pse.tile([DL, 1024], F32, tag="pse")
                    for w in range(2):
                        nc.tensor.matmul(
                            pe[:, w * 512 : (w + 1) * 512],
                            we0_sb[:],
                            src[:, (g * 2 + w) * 512 : (g * 2 + w + 1) * 512],
                            skip_group_check=True,
                        )
                    nc.scalar.activation(
                        e_sb[:, g * 1024 : (g + 1) * 1024],
                        pe[:],
                        AF.Relu,
                        bias=bias_sb[:, 0:1],
                        scale=1.0,
                    )
            else:
                wcol = (l - 1) * DL
                for half in range(2):
                    q = half * DP
                    lhsT = wedup_sb[q : q + DP, wcol : wcol + DL]
                    pe = pse.tile([DL, 1024], F32, tag="pse")
                    for w in range(2):
                        nc.tensor.matmul(
                            pe[:, w * 512 : (w + 1) * 512],
                            lhsT,
                            src[q : q + DP, w * 512 : (w + 1) * 512],
                            tile_position=(q, 0),
                            skip_group_check=True,
                        )
                    nc.scalar.activation(
                        e_sb[:, half * 1024 : (half + 1) * 1024],
                        pe[:],
                        AF.Relu,
                        bias=bias_sb[:, bcol : bcol + 1],
                        scale=1.0,
                    )
            return e_sb

        def stage_shrink_copy(l, e_sb):
            """Shrink into two 1-bank PSUM window tiles (halves col-tiled),
            copy each window -> h_sb fp16, fill halos (GpSimd)."""
            ws_l = ws_sb[:, l * DP : (l + 1) * DP]
            h_sb = hp.tile([2 * DP, HW], F16)
            pcs = []
            for w in range(2):
                pc = psc.tile([2 * DP, 512], F32, tag="psc")
                nc.tensor.matmul(
                    pc[0:DP, :],
                    ws_l,
                    e_sb[:, w * 512 : (w + 1) * 512],
                    tile_position=(0, 0),
                    start=True,
                    stop=False,
                    skip_group_check=True,
                )
                nc.tensor.matmul(
                    pc[DP : 2 * DP, :],
                    ws_l,
                    e_sb[:, H + w * 512 : H + (w + 1) * 512],
                    tile_position=(0, DP),
                    start=True,
                    stop=False,
                    skip_group_check=True,
                )
                nc.scalar.activation(
                    h_sb[:, HALO_L + w * 512 : HALO_L + (w + 1) * 512],
                    pc[:],
                    AF.Copy,
                    bias=0.0,
                    scale=1.0,
                )
                pcs.append(pc)
            # halos on GpSimd: zeros at the outer edges, cross-half copies
            nc.gpsimd.tensor_copy(h_sb[0:DP, 0:HALO_L], zero_sb[0:DP, :])
            nc.gpsimd.tensor_copy(
                h_sb[DP : 2 * DP, HALO_L + H : HW], zero_sb[DP : 2 * DP, 0:RO]
            )
            nc.gpsimd.tensor_copy(
                h_sb[DP : 2 * DP, 0:HALO_L], h_sb[0:DP, H : H + HALO_L]
            )
            nc.gpsimd.tensor_copy(
                h_sb[0:DP, HALO_L + H : HW],
                h_sb[DP : 2 * DP, HALO_L : HALO_L + 1],
            )
            return pcs, h_sb

        def stage_conv(l, pcs, h_sb, o_prev):
            """PE taps accumulate into the pcs windows; Act taps make scaled
            copies; DVE chain sums everything; per-window merge o = s + pcs."""
            pe_taps = PE_TAPS_L[l]
            for i, j in enumerate(pe_taps):
                for w in range(2):
                    a = j - 9 + HALO_L + w * 512
                    nc.tensor.matmul(
                        pcs[w][:, :],
                        diag(l, i),
                        h_sb[:, a : a + 512],
                        start=False,
                        stop=(i == len(pe_taps) - 1),
                        skip_group_check=True,
                    )

            # Act + GpSimd taps: scaled shifted copies (per-partition scale)
            act_tmps = []
            for j in ACT_TAPS_L[l]:
                a = j - 9 + HALO_L
                tmp = tp_.tile([2 * DP, H], F16, tag="t")
                nc.scalar.activation(
                    tmp[:], h_sb[:, a : a + H], AF.Copy, bias=0.0, scale=tap(l, j)
                )
                act_tmps.append(tmp)

            # DVE chain over the full H width
            s_sb = sp_.tile([2 * DP, H], F16)
            first = True
            for j in DVE_TAPS_L[l]:
                a = j - 9 + HALO_L
                src = h_sb[:, a : a + H]
                if first and o_prev is None:
                    nc.vector.tensor_scalar_mul(s_sb[:], src, tap(l, j))
                elif first:
                    nc.vector.scalar_tensor_tensor(
                        s_sb[:], src, tap(l, j), o_prev[:], OP.mult, OP.add
                    )
                else:
                    nc.vector.scalar_tensor_tensor(
                        s_sb[:], src, tap(l, j), s_sb[:], OP.mult, OP.add
                    )
                first = False
            for tmp in act_tmps:
                nc.vector.tensor_tensor(s_sb[:], s_sb[:], tmp[:], OP.add)

            o_new = op_.tile([2 * DP, H], F16)
            for w in range(2):
                ws_ = slice(w * 512, (w + 1) * 512)
                nc.vector.scalar_tensor_tensor(
                    o_new[:, ws_], pcs[w][:], 1.0, s_sb[:, ws_], OP.mult, OP.add
                )
            return o_new

        def emit_unit(l, state, nstreams):
            """One unit for all channel-streams with staggered A/B emission."""
            pend = {}
            for c in range(nstreams):
                e_sb = stage_expand(l, state[c])
                pend[c] = stage_shrink_copy(l, e_sb)
                if c >= 1:
                    pcs, h_sb = pend.pop(c - 1)
                    state[c - 1] = stage_conv(
                        l, pcs, h_sb, None if l == 0 else state[c - 1]
                    )
            pcs, h_sb = pend.pop(nstreams - 1)
            state[nstreams - 1] = stage_conv(
                l, pcs, h_sb, None if l == 0 else state[nstreams - 1]
            )

        def emit_pool_decode(b, f_tiles):
            """Channel maxpool (GpSimd) + decoder (PE/Act) + output DMA."""
            m01 = pp.tile([DL, T], F16, tag="m")
            m23 = pp.tile([DL, T], F16, tag="m")
            nc.vector.tensor_tensor(m01[:], f_tiles[0][:], f_tiles[1][:], OP.max)
            nc.vector.tensor_tensor(m23[:], f_tiles[2][:], f_tiles[3][:], OP.max)
            pooled = pp.tile([DL, T], F16, tag="pool")
            nc.vector.tensor_tensor(pooled[:], m01[:], m23[:], OP.max)

            out_sb = osb.tile([S, T], F32)
            for g in range(2):
                pd = pse.tile([S, 1024], F32, tag="pse")
                for w in range(2):
                    nc.tensor.matmul(
                        pd[:, w * 512 : (w + 1) * 512],
                        wd_sb[:],
                        pooled[:, (g * 2 + w) * 512 : (g * 2 + w + 1) * 512],
                        skip_group_check=True,
                    )
                nc.scalar.activation(
                    out_sb[:, g * 1024 : (g + 1) * 1024],
                    pd[:],
                    AF.Identity,
                    bias=bd_sb[:, 0:1],
                    scale=1.0,
                )
            nc.sync.dma_start(out=out_d[b], in_=out_sb[:])

        # process batches in pairs: 8 independent channel-streams interleave
        # per unit so every engine always has unrelated work between the
        # dependent stages of any one stream.
        NB = 2  # batches per pair
        NS = NB * C  # streams per pair
        pending_pools = []  # [(batch, f_tiles), ...] awaiting pool+decode
        for pair in range(BPC // NB):
            bs = [pair * NB + i for i in range(NB)]
            x_tiles = []
            for c in range(C):
                for b in bs:
                    x_sb = xp.tile([F, T], F16)
                    nc.sync.dma_start(out=x_sb[:], in_=xt_d[b * C + c])
                    x_tiles.append(x_sb)
            if pair == 0:
                # big PE-tap weight load queued behind first inputs so the
                # first expands are not starved at startup
                nc.sync.dma_start(out=diag_sb[:], in_=diag_d[:])

            state = {s: x_tiles[s] for s in range(NS)}
            emit_unit(0, state, NS)
            for pp_args in pending_pools:
                emit_pool_decode(*pp_args)
            pending_pools = []
            for l in range(1, L):
                emit_unit(l, state, NS)

            # final expand (We2): bias slot 5, weight col 4
            f_tiles = [[] for _ in bs]
            for s in range(NS):
                e_sb = fp.tile([DL, T], F16)
                for half in range(2):
                    q = half * DP
                    lhsT = wedup_sb[q : q + DP, 4 * DL : 5 * DL]
                    pe = pse.tile([DL, 1024], F32, tag="pse")
                    for w in range(2):
                        nc.tensor.matmul(
                            pe[:, w * 512 : (w + 1) * 512],
                            lhsT,
                            state[s][q : q + DP, w * 512 : (w + 1) * 512],
                            tile_position=(q, 0),
                            skip_group_check=True,
                        )
                    nc.scalar.activation(
                        e_sb[:, half * 1024 : (half + 1) * 1024],
                        pe[:],
                        AF.Relu,
                        bias=bias_sb[:, L : L + 1],
                        scale=1.0,
                    )
                f_tiles[s % NB].append(e_sb)
            pending_pools = [(bs[i], f_tiles[i]) for i in range(NB)]

        for pp_args in pending_pools:
            emit_pool_decode(*pp_args)

    nc.compile()
    return nc


_NC = None


def get_nc():
    global _NC
    if _NC is None:
        _NC = build_nc()
    return _NC


def prep_in_maps(x, We0, be0, Ws0, wl0, wr0, We, be, Ws, wl, wr, We2, be2, Wd, bd):
    xt = np.ascontiguousarray(
        x.transpose(0, 2, 3, 1), dtype=np.float16
    )  # [B,C,F,T] fp16

    wedup = np.stack(
        [np.concatenate([w, w], axis=0) for w in [We[0], We[1], We[2], We[3], We2]]
    ).astype(np.float16)  # [L, 128, 128]
    ws_all = np.stack([Ws0, Ws[0], Ws[1], Ws[2], Ws[3]]).astype(np.float16)
    biases = np.stack([be0, be[0], be[1], be[2], be[3], be2], axis=1).astype(
        np.float32
    )  # [128, 6]

    wl_full = np.concatenate([wl0[None], wl], axis=0)  # [L, 10, 64]
    wr_full = np.concatenate([wr0[None], wr], axis=0)  # [L, 1, 64]
    taps64 = np.concatenate([wl_full, wr_full], axis=1)  # [L, 11, 64]; col j = delta j-9
    # NO +1 identity fold: the shrink PSUM already contributes h with coeff 1.
    taps = np.tile(
        taps64.transpose(2, 0, 1).reshape(DP, L * 11), (2, 1)
    )  # [128, 55], col = l*11 + j
    taps = np.ascontiguousarray(taps, dtype=np.float32)

    # block-diag tap matrices for the PE conv taps
    diag = np.zeros((L, NPE_MAX, 2 * DP, 2 * DP), np.float32)
    for l in range(L):
        for i, j in enumerate(PE_TAPS_L[l]):
            np.fill_diagonal(diag[l, i], np.tile(taps64[l, j, :], 2))
    diag2 = diag.transpose(2, 0, 1, 3).reshape(2 * DP, L * NPE_MAX * 2 * DP)
    diag2 = np.ascontiguousarray(diag2).astype(np.float16)

    shared = dict(
        we0=np.ascontiguousarray(We0, dtype=np.float16),
        wedup=wedup,
        ws=ws_all,
        wd=np.ascontiguousarray(Wd, dtype=np.float16),
        biases=np.ascontiguousarray(biases),
        taps=taps,
        diag=diag2,
        bd=np.ascontiguousarray(bd.reshape(S, 1), dtype=np.float32),
    )
    in_maps = []
    for k in range(NCORES):
        xs = xt[k * BPC : (k + 1) * BPC].reshape(SEQ, F, T)
        m = dict(shared)
        m["xt"] = np.ascontiguousarray(xs)
        in_maps.append(m)
    return in_maps


def postprocess(results):
    full = np.concatenate([r["out"] for r in results], axis=0)  # [B, S, T]
    return np.ascontiguousarray(full.transpose(0, 2, 1))  # [B, T, S]


def kernel(**inputs):
    nc = get_nc()
    in_maps = prep_in_maps(**inputs)
    res = run_bass_kernel_spmd(nc, in_maps, core_ids=list(range(NCORES)))
    return postprocess(res.results)


# revision 24
# speedup vs baseline: 1.0573x; 1.0573x over previous
"""Trainium2 Bass kernel for nn_FSMNSeleNetV3 (FSMN stack + channel maxpool + decoder).

Self-contained: hardcodes all shapes from the problem spec and only imports
numpy + the concourse stack from /opt/trn_rl_repo.

Sharding: pure data parallel over batch. Each of the 8 cores processes 4
batches x 4 channels = 16 independent sequences of T=2048 tokens.

v3 design:
- fp16 everywhere on the matmul path (fp32r lowers to fp32_mode=HIGH which is
  ~2x slower per column AND disables FWL; 16-bit gets 1 cycle/row + fast
  weight load, and fp16's 11-bit mantissa keeps precision).
- The shrink writes both T-halves into window-granular 1-bank PSUM tiles via
  column tiling (tile_position (0,0)/(0,64)) -> the halves-stacked h layout.
- FSMN conv: taps delta -9..-2 as 128x128 block-diag fp16 matmuls on the PE
  accumulating into the shrink PSUM (DVE scalar_tensor_tensor only has a
  1x-mode uop, so the PE is the cheapest tap engine); taps delta -1,0,+1 as
  a DVE chain; per-window merge (s + pcs) on the DVE.
- Halo copies and the channel maxpool run on the otherwise-idle GpSimd.
- 4-channel software pipelining per unit plus cross-batch overlap (pool +
  decoder of batch b are emitted after batch b+1's unit-0 stages) to keep
  the PE saturated so the HAM clock gate stays at 2.4 GHz.
"""

import sys

sys.path.insert(0, "/opt/trn_rl_repo")
from contextlib import ExitStack

import numpy as np

import concourse.bass as bass  # noqa: F401
import concourse.mybir as mybir
import concourse.tile as tile
from concourse import bacc
from concourse.bass_utils import run_bass_kernel_spmd

F32 = mybir.dt.float32
F16 = mybir.dt.float16
AF = mybir.ActivationFunctionType
OP = mybir.AluOpType

NCORES = 8
B, T, C, F = 32, 2048, 4, 120
DL, DP, L, LO, RO, S = 128, 64, 5, 10, 1, 5
BPC = B // NCORES  # batches per core
SEQ = BPC * C  # sequences per core
H = T // 2  # half-sequence length (halves stacked on partitions)
HALO_L = LO - 1  # 9 left halo columns
HW = H + HALO_L + RO  # h buffer width: 1034

# conv tap split per unit: tap index j = delta + 9, delta in [-9..+1].
# PE taps run as block-diag matmuls accumulating into the shrink PSUM; Act
# taps as scaled copies (per-partition scale AP) summed by the DVE at 2x TT
# rate; DVE taps as an stt chain whose seed carries the residual.
PE_TAPS_L = {l: [0, 1, 2, 3, 4, 5, 6] for l in range(L)}
ACT_TAPS_L = {l: [7, 10] for l in range(L)}
DVE_TAPS_L = {l: [8, 9] for l in range(L)}
NPE_MAX = 7  # diag table stride (max PE taps of any unit)


def build_nc():
    nc = bacc.Bacc("TRN2", target_bir_lowering=False, debug=False, num_devices=NCORES)

    xt_d = nc.dram_tensor("xt", [SEQ, F, T], F16, kind="ExternalInput")
    we0_d = nc.dram_tensor("we0", [F, DL], F16, kind="ExternalInput")
    wedup_d = nc.dram_tensor("wedup", [L, 2 * DP, DL], F16, kind="ExternalInput")
    ws_d = nc.dram_tensor("ws", [L, DL, DP], F16, kind="ExternalInput")
    wd_d = nc.dram_tensor("wd", [DL, S], F16, kind="ExternalInput")
    biases_d = nc.dram_tensor("biases", [DL, L + 1], F32, kind="ExternalInput")
    taps_d = nc.dram_tensor("taps", [2 * DP, L * 11], F32, kind="ExternalInput")
    diag_d = nc.dram_tensor(
        "diag", [2 * DP, L * NPE_MAX * 2 * DP], F16, kind="ExternalInput"
    )
    bd_d = nc.dram_tensor("bd", [S, 1], F32, kind="ExternalInput")
    out_d = nc.dram_tensor("out", [BPC, S, T], F32, kind="ExternalOutput")

    with tile.TileContext(nc) as tc, ExitStack() as ctx:
        wp = ctx.enter_context(tc.tile_pool(name="weights", bufs=1))
        xp = ctx.enter_context(tc.tile_pool(name="x", bufs=9))
        ep = ctx.enter_context(tc.tile_pool(name="e", bufs=5))
        hp = ctx.enter_context(tc.tile_pool(name="h", bufs=4))
        sp_ = ctx.enter_context(tc.tile_pool(name="s", bufs=4))
        tp_ = ctx.enter_context(tc.tile_pool(name="tmp", bufs=10))
        op_ = ctx.enter_context(tc.tile_pool(name="o", bufs=10))
        fp = ctx.enter_context(tc.tile_pool(name="f", bufs=10))
        pp = ctx.enter_context(tc.tile_pool(name="pooled", bufs=4))
        osb = ctx.enter_context(tc.tile_pool(name="osb", bufs=1))
        pse = ctx.enter_context(tc.tile_pool(name="pse", bufs=2, space="PSUM"))
        psc = ctx.enter_context(tc.tile_pool(name="psc", bufs=4, space="PSUM"))

        # --- weights / constants (loaded once) ---
        we0_sb = wp.tile([F, DL], F16)
        nc.sync.dma_start(out=we0_sb[:], in_=we0_d[:])
        wedup_sb = wp.tile([2 * DP, L * DL], F16)
        ws_sb = wp.tile([DL, L * DP], F16)
        for l in range(L):
            nc.sync.dma_start(out=wedup_sb[:, l * DL : (l + 1) * DL], in_=wedup_d[l])
            nc.sync.dma_start(out=ws_sb[:, l * DP : (l + 1) * DP], in_=ws_d[l])
        wd_sb = wp.tile([DL, S], F16)
        nc.sync.dma_start(out=wd_sb[:], in_=wd_d[:])
        bias_sb = wp.tile([DL, L + 1], F32)
        nc.sync.dma_start(out=bias_sb[:], in_=biases_d[:])
        taps_sb = wp.tile([2 * DP, L * 11], F32)
        nc.sync.dma_start(out=taps_sb[:], in_=taps_d[:])
        diag_sb = wp.tile([2 * DP, L * NPE_MAX * 2 * DP], F16)
        bd_sb = wp.tile([S, 1], F32)
        nc.sync.dma_start(out=bd_sb[:], in_=bd_d[:])
        zero_sb = wp.tile([2 * DP, HALO_L], F16)
        nc.gpsimd.memset(zero_sb[:], 0.0)

        def tap(l, j):
            return taps_sb[:, l * 11 + j : l * 11 + j + 1]

        def diag(l, i):
            col = (l * NPE_MAX + i) * 2 * DP
            return diag_sb[:, col : col + 2 * DP]

        def stage_expand(l, src):
            """Expand + relu -> e_sb [128, T] fp16 (plain-T layout).

            l==0: src is x_sb [120, T].  1<=l<=4: src is o [2x64 halves, H].
            l==5: final expand with We2."""
            e_sb = ep.tile([DL, T], F16)
            bcol = 0 if l == 0 else l
            if l == 0:
                for g in range(2):
                    pe = pse.tile([DL, 1024], F32, tag="pse")
                    for w in range(2):
                        nc.tensor.matmul(
                            pe[:, w * 512 : (w + 1) * 512],
                            we0_sb[:],
                            src[:, (g * 2 + w) * 512 : (g * 2 + w + 1) * 512],
                            skip_group_check=True,
                        )
                    nc.scalar.activation(
                        e_sb[:, g * 1024 : (g + 1) * 1024],
                        pe[:],
                        AF.Relu,
                        bias=bias_sb[:, 0:1],
                        scale=1.0,
                    )
            else:
                wcol = (l - 1) * DL
                for half in range(2):
                    q = half * DP
                    lhsT = wedup_sb[q : q + DP, wcol : wcol + DL]
                    pe = pse.tile([DL, 1024], F32, tag="pse")
                    for w in range(2):
                        nc.tensor.matmul(
                            pe[:, w * 512 : (w + 1) * 512],
                            lhsT,
                            src[q : q + DP, w * 512 : (w + 1) * 512],
                            tile_position=(q, 0),
                            skip_group_check=True,
                        )
                    nc.scalar.activation(
                        e_sb[:, half * 1024 : (half + 1) * 1024],
                        pe[:],
                        AF.Relu,
                        bias=bias_sb[:, bcol : bcol + 1],
                        scale=1.0,
                    )
            return e_sb

        def stage_shrink_copy(l, e_sb):
            """Shrink into two 1-bank PSUM window tiles (halves col-tiled),
            copy each window -> h_sb fp16, fill halos (GpSimd)."""
            ws_l = ws_sb[:, l * DP : (l + 1) * DP]
            h_sb = hp.tile([2 * DP, HW], F16)
            pcs = []
            for w in range(2):
                pc = psc.tile([2 * DP, 512], F32, tag="psc")
                nc.tensor.matmul(
                    pc[0:DP, :],
                    ws_l,
                    e_sb[:, w * 512 : (w + 1) * 512],
                    tile_position=(0, 0),
                    start=True,
                    stop=False,
                    skip_group_check=True,
                )
                nc.tensor.matmul(
                    pc[DP : 2 * DP, :],
                    ws_l,
                    e_sb[:, H + w * 512 : H + (w + 1) * 512],
                    tile_position=(0, DP),
                    start=True,
                    stop=False,
                    skip_group_check=True,
                )
                nc.scalar.activation(
                    h_sb[:, HALO_L + w * 512 : HALO_L + (w + 1) * 512],
                    pc[:],
                    AF.Copy,
                    bias=0.0,
                    scale=1.0,
                )
                pcs.append(pc)
            # halos on GpSimd: zeros at the outer edges, cross-half copies
            nc.gpsimd.tensor_copy(h_sb[0:DP, 0:HALO_L], zero_sb[0:DP, :])
            nc.gpsimd.tensor_copy(
                h_sb[DP : 2 * DP, HALO_L + H : HW], zero_sb[DP : 2 * DP, 0:RO]
            )
            nc.gpsimd.tensor_copy(
                h_sb[DP : 2 * DP, 0:HALO_L], h_sb[0:DP, H : H + HALO_L]
            )
            nc.gpsimd.tensor_copy(
                h_sb[0:DP, HALO_L + H : HW],
                h_sb[DP : 2 * DP, HALO_L : HALO_L + 1],
            )
            return pcs, h_sb

        def stage_conv(l, pcs, h_sb, o_prev):
            """PE taps accumulate into the pcs windows; Act taps make scaled
            copies; DVE chain sums everything; per-window merge o = s + pcs."""
            pe_taps = PE_TAPS_L[l]
            for i, j in enumerate(pe_taps):
                for w in range(2):
                    a = j - 9 + HALO_L + w * 512
                    nc.tensor.matmul(
                        pcs[w][:, :],
                        diag(l, i),
                        h_sb[:, a : a + 512],
                        start=False,
                        stop=(i == len(pe_taps) - 1),
                        skip_group_check=True,
                    )

            # Act + GpSimd taps: scaled shifted copies (per-partition scale)
            act_tmps = []
            for j in ACT_TAPS_L[l]:
                a = j - 9 + HALO_L
                tmp = tp_.tile([2 * DP, H], F16, tag="t")
                nc.scalar.activation(
                    tmp[:], h_sb[:, a : a + H], AF.Copy, bias=0.0, scale=tap(l, j)
                )
                act_tmps.append(tmp)

            # DVE chain over the full H width
            s_sb = sp_.tile([2 * DP, H], F16)
            first = True
            for j in DVE_TAPS_L[l]:
                a = j - 9 + HALO_L
                src = h_sb[:, a : a + H]
                if first and o_prev is None:
                    nc.vector.tensor_scalar_mul(s_sb[:], src, tap(l, j))
                elif first:
                    nc.vector.scalar_tensor_tensor(
                        s_sb[:], src, tap(l, j), o_prev[:], OP.mult, OP.add
                    )
                else:
                    nc.vector.scalar_tensor_tensor(
                        s_sb[:], src, tap(l, j), s_sb[:], OP.mult, OP.add
                    )
                first = False
            for tmp in act_tmps:
                nc.vector.tensor_tensor(s_sb[:], s_sb[:], tmp[:], OP.add)

            o_new = op_.tile([2 * DP, H], F16)
            for w in range(2):
                ws_ = slice(w * 512, (w + 1) * 512)
                nc.vector.scalar_tensor_tensor(
                    o_new[:, ws_], pcs[w][:], 1.0, s_sb[:, ws_], OP.mult, OP.add
                )
            return o_new

        def emit_unit(l, state, nstreams):
            """One unit for all channel-streams with staggered A/B emission."""
            pend = {}
            for c in range(nstreams):
                e_sb = stage_expand(l, state[c])
                pend[c] = stage_shrink_copy(l, e_sb)
                if c >= 1:
                    pcs, h_sb = pend.pop(c - 1)
                    state[c - 1] = stage_conv(
                        l, pcs, h_sb, None if l == 0 else state[c - 1]
                    )
            pcs, h_sb = pend.pop(nstreams - 1)
            state[nstreams - 1] = stage_conv(
                l, pcs, h_sb, None if l == 0 else state[nstreams - 1]
            )

        def emit_pool_decode(b, f_tiles):
            """Channel maxpool (GpSimd) + decoder (PE/Act) + output DMA."""
            m01 = pp.tile([DL, T], F16, tag="m")
            m23 = pp.tile([DL, T], F16, tag="m")
            nc.vector.tensor_tensor(m01[:], f_tiles[0][:], f_tiles[1][:], OP.max)
            nc.vector.tensor_tensor(m23[:], f_tiles[2][:], f_tiles[3][:], OP.max)
            pooled = pp.tile([DL, T], F16, tag="pool")
            nc.vector.tensor_tensor(pooled[:], m01[:], m23[:], OP.max)

            out_sb = osb.tile([S, T], F32)
            for g in range(2):
                pd = pse.tile([S, 1024], F32, tag="pse")
                for w in range(2):
                    nc.tensor.matmul(
                        pd[:, w * 512 : (w + 1) * 512],
                        wd_sb[:],
                        pooled[:, (g * 2 + w) * 512 : (g * 2 + w + 1) * 512],
                        skip_group_check=True,
                    )
                nc.scalar.activation(
                    out_sb[:, g * 1024 : (g + 1) * 1024],
                    pd[:],
                    AF.Identity,
                    bias=bd_sb[:, 0:1],
                    scale=1.0,
                )
            nc.sync.dma_start(out=out_d[b], in_=out_sb[:])

        # process batches in pairs: 8 independent channel-streams interleave
        # per unit so every engine always has unrelated work between the
        # dependent stages of any one stream.
        NB = 1  # batches per pair (4 streams measured best)
        NS = NB * C  # streams per pair
        pending_pools = []  # [(batch, f_tiles), ...] awaiting pool+decode
        for pair in range(BPC // NB):
            bs = [pair * NB + i for i in range(NB)]
            x_tiles = []
            for c in range(C):
                for b in bs:
                    x_sb = xp.tile([F, T], F16)
                    nc.sync.dma_start(out=x_sb[:], in_=xt_d[b * C + c])
                    x_tiles.append(x_sb)
            if pair == 0:
                # big PE-tap weight load queued behind first inputs so the
                # first expands are not starved at startup
                nc.sync.dma_start(out=diag_sb[:], in_=diag_d[:])

            state = {s: x_tiles[s] for s in range(NS)}
            emit_unit(0, state, NS)
            for pp_args in pending_pools:
                emit_pool_decode(*pp_args)
            pending_pools = []
            for l in range(1, L):
                emit_unit(l, state, NS)

            # final expand (We2): bias slot 5, weight col 4
            f_tiles = [[] for _ in bs]
            for s in range(NS):
                e_sb = fp.tile([DL, T], F16)
                for half in range(2):
                    q = half * DP
                    lhsT = wedup_sb[q : q + DP, 4 * DL : 5 * DL]
                    pe = pse.tile([DL, 1024], F32, tag="pse")
                    for w in range(2):
                        nc.tensor.matmul(
                            pe[:, w * 512 : (w + 1) * 512],
                            lhsT,
                            state[s][q : q + DP, w * 512 : (w + 1) * 512],
                            tile_position=(q, 0),
                            skip_group_check=True,
                        )
                    nc.scalar.activation(
                        e_sb[:, half * 1024 : (half + 1) * 1024],
                        pe[:],
                        AF.Relu,
                        bias=bias_sb[:, L : L + 1],
                        scale=1.0,
                    )
                f_tiles[s % NB].append(e_sb)
            pending_pools = [(bs[i], f_tiles[i]) for i in range(NB)]

        for pp_args in pending_pools:
            emit_pool_decode(*pp_args)

    nc.compile()
    return nc


_NC = None


def get_nc():
    global _NC
    if _NC is None:
        _NC = build_nc()
    return _NC


def prep_in_maps(x, We0, be0, Ws0, wl0, wr0, We, be, Ws, wl, wr, We2, be2, Wd, bd):
    xt = np.ascontiguousarray(
        x.transpose(0, 2, 3, 1), dtype=np.float16
    )  # [B,C,F,T] fp16

    wedup = np.stack(
        [np.concatenate([w, w], axis=0) for w in [We[0], We[1], We[2], We[3], We2]]
    ).astype(np.float16)  # [L, 128, 128]
    ws_all = np.stack([Ws0, Ws[0], Ws[1], Ws[2], Ws[3]]).astype(np.float16)
    biases = np.stack([be0, be[0], be[1], be[2], be[3], be2], axis=1).astype(
        np.float32
    )  # [128, 6]

    wl_full = np.concatenate([wl0[None], wl], axis=0)  # [L, 10, 64]
    wr_full = np.concatenate([wr0[None], wr], axis=0)  # [L, 1, 64]
    taps64 = np.concatenate([wl_full, wr_full], axis=1)  # [L, 11, 64]; col j = delta j-9
    # NO +1 identity fold: the shrink PSUM already contributes h with coeff 1.
    taps = np.tile(
        taps64.transpose(2, 0, 1).reshape(DP, L * 11), (2, 1)
    )  # [128, 55], col = l*11 + j
    taps = np.ascontiguousarray(taps, dtype=np.float32)

    # block-diag tap matrices for the PE conv taps
    diag = np.zeros((L, NPE_MAX, 2 * DP, 2 * DP), np.float32)
    for l in range(L):
        for i, j in enumerate(PE_TAPS_L[l]):
            np.fill_diagonal(diag[l, i], np.tile(taps64[l, j, :], 2))
    diag2 = diag.transpose(2, 0, 1, 3).reshape(2 * DP, L * NPE_MAX * 2 * DP)
    diag2 = np.ascontiguousarray(diag2).astype(np.float16)

    shared = dict(
        we0=np.ascontiguousarray(We0, dtype=np.float16),
        wedup=wedup,
        ws=ws_all,
        wd=np.ascontiguousarray(Wd, dtype=np.float16),
        biases=np.ascontiguousarray(biases),
        taps=taps,
        diag=diag2,
        bd=np.ascontiguousarray(bd.reshape(S, 1), dtype=np.float32),
    )
    in_maps = []
    for k in range(NCORES):
        xs = xt[k * BPC : (k + 1) * BPC].reshape(SEQ, F, T)
        m = dict(shared)
        m["xt"] = np.ascontiguousarray(xs)
        in_maps.append(m)
    return in_maps


def postprocess(results):
    full = np.concatenate([r["out"] for r in results], axis=0)  # [B, S, T]
    return np.ascontiguousarray(full.transpose(0, 2, 1))  # [B, T, S]


def kernel(**inputs):
    nc = get_nc()
    in_maps = prep_in_maps(**inputs)
    res = run_bass_kernel_spmd(nc, in_maps, core_ids=list(range(NCORES)))
    return postprocess(res.results)
